# revision 12
# baseline (speedup 1.0000x reference)
"""Trainium2 Bass kernel for nn_Encoder_60112362275055 (GRU with skip connections).

B=64, T=512, X=256, H=1024, skip_size=5. Output = 2 * h_{T-1}  -> [64, 1024].

Strategy: data-parallel over batch (8 cores x B_local=8), ZERO cross-core
communication.  The skip structure (w1[t]==0 drops the h_{t-1} dependency)
turns the scan into a DAG whose critical path is ~88 topological levels
(avg width ~6).  All timesteps in one level are batched into a single
[8w, 1024] @ [1024, 3072] float32r matmul per core per level.
"""

import sys

import numpy as np

sys.path.insert(0, "/opt/trn_rl_repo")

import concourse.bacc as bacc
import concourse.bass as bass
import concourse.mybir as mybir
from concourse import tile
from concourse.bass_utils import run_bass_kernel_spmd

B, T, X, H = 64, 512, 256, 1024
SKIP = 5
NCORES = 8
BL = B // NCORES  # 8
G3 = 3 * H  # 3072
NB = G3 // 512  # 6 psum banks per level
KC = H // 128  # 8 K-chunks
WMAX = 9  # max level width (recomputed per input in kernel())
S_HIST = 96  # circular history slots

f32 = mybir.dt.float32
f32r = mybir.dt.float32r
AF = mybir.ActivationFunctionType


def _skip_plan(T, skip_size):
    slots = np.zeros(T, np.int32)
    use_zero = np.zeros(T, np.float32)
    for i in range(T):
        if i < skip_size:
            if 2 * i < skip_size:
                use_zero[i] = 1.0
            else:
                slots[i] = (skip_size - i) - 1
        else:
            if i - skip_size < skip_size:
                use_zero[i] = 1.0
            else:
                slots[i] = 2 * skip_size - 1
    return slots, use_zero


def _plan(w1, w2):
    """Topological levels of the recurrence DAG + per-node dependency spec."""
    slots, use_zero = _skip_plan(T, SKIP)
    d1 = np.full(T, -1, np.int64)  # h_{t-1} dep (coeff w1)
    d2 = np.full(T, -1, np.int64)  # skip dep (coeff w2*(1-uz))
    for t in range(T):
        if w1[t] == 1 and t - 1 >= 0:
            d1[t] = t - 1
        if w2[t] == 1 and use_zero[t] == 0.0 and t - 1 - slots[t] >= 0:
            d2[t] = t - 1 - slots[t]
    level = np.zeros(T, np.int64)
    for t in range(T):
        deps = [d for d in (d1[t], d2[t]) if d >= 0]
        level[t] = 1 + max((level[d] for d in deps), default=-1)
    nlev = int(level.max()) + 1
    levels = [[] for _ in range(nlev)]
    for t in range(T):
        levels[level[t]].append(t)
    order = [t for lv in levels for t in lv]
    slot_of = {t: s for s, t in enumerate(order)}

    # circular-history safety: the writer of slot s+S_HIST must come after the
    # last consumer of slot s.
    lev_of_slot = [level[order[s]] for s in range(T)]
    last_use = {}
    for t in range(T):
        for d in (d1[t], d2[t]):
            if d >= 0:
                s = slot_of[d]
                last_use[s] = max(last_use.get(s, -1), int(level[t]))
    for s in range(T):
        s2 = s + S_HIST
        if s2 < T and s in last_use:
            assert lev_of_slot[s2] > last_use[s], (s, s2, "S_HIST too small")
    return levels, order, slot_of, d1, d2


def _build(levels, slot_of, d1, d2):
    """Emit the bass program (identical on all 8 cores)."""
    nc = bacc.Bacc(None)

    xs_d = nc.dram_tensor("xs", [2, 128, T * BL], f32r, kind="ExternalInput")
    wih_d = nc.dram_tensor("wih", [2, 128, G3], f32r, kind="ExternalInput")
    whh_d = nc.dram_tensor("whh", [KC, 128, G3], f32r, kind="ExternalInput")
    biasg_d = nc.dram_tensor("biasg", [128, G3], f32, kind="ExternalInput")
    bias2_d = nc.dram_tensor("bias2", [128, H], f32, kind="ExternalInput")
    ident_d = nc.dram_tensor("ident", [128, 128], f32, kind="ExternalInput")
    zeros_d = nc.dram_tensor("zeros", [128, 64], f32r, kind="ExternalInput")
    out_d = nc.dram_tensor("out", [BL, H], f32, kind="ExternalOutput")
    xi_d = nc.dram_tensor("xi_scratch", [T * BL, G3], f32)

    NLEV = len(levels)

    def hbl_chunk(hbl_ap, w, c):
        """lhsT for K-chunk c (chunk-major layout): contiguous [128, w*8]."""
        return hbl_ap[:, c * WMAX * BL : c * WMAX * BL + w * BL]

    def hbl_node(hbl_ap, i):
        """blend dst for node i: strided [128, KC, BL]."""
        return hbl_ap.rearrange("p (c n b) -> p c n b", n=WMAX, b=BL)[:, :, i, :]

    def hist_cols(hist_ap, sm, w, c):
        """history store cols (sm+i)*64 + c*8 + b -> [128, w, 8] (no wrap)."""
        return hist_ap.rearrange("p (s c b) -> p s c b", c=KC, b=BL)[:, sm : sm + w, c, :]

    with tile.TileContext(nc) as tc:
        with (
            tc.tile_pool(name="consts", bufs=1) as cpool,
            tc.tile_pool(name="ps", bufs=6, space="PSUM") as pspool,
            tc.tile_pool(name="pst", bufs=2, space="PSUM") as ptpool,
        ):
            ident = cpool.tile([128, 128], f32)
            nc.sync.dma_start(ident[:], ident_d[:])
            zeros = cpool.tile([128, 64], f32r)
            nc.sync.dma_start(zeros[:], zeros_d[:])

            # ---------- phase 1: xi = x @ W_ih.T + bias (level-sorted rows) ----------
            with tc.tile_pool(name="ph1", bufs=1) as p1, tc.tile_pool(name="ph1b", bufs=3) as p1b:
                wih = p1.tile([128, 2 * G3], f32r)
                nc.sync.dma_start(
                    wih[:].rearrange("p (k f) -> p k f", k=2),
                    wih_d.rearrange("k p f -> p k f"),
                )
                xs = p1.tile([128, 2 * T * BL], f32r)
                nc.sync.dma_start(
                    xs[:].rearrange("p (k f) -> p k f", k=2),
                    xs_d.rearrange("k p f -> p k f"),
                )
                biasg = p1.tile([128, G3], f32)
                nc.sync.dma_start(biasg[:], biasg_d[:])
                MT = T * BL // 128  # 32 M-tiles
                for m in range(MT):
                    for nb in range(NB):
                        ps = pspool.tile([128, 512], f32, tag="ps")
                        for k in range(2):
                            nc.tensor.matmul(
                                ps[:],
                                xs[:, k * T * BL + m * 128 : k * T * BL + (m + 1) * 128],
                                wih[:, k * G3 + nb * 512 : k * G3 + (nb + 1) * 512],
                                start=(k == 0),
                                stop=(k == 1),
                            )
                        sb = p1b.tile([128, 512], f32, tag="xsb")
                        nc.vector.tensor_add(
                            sb[:], ps[:], biasg[:, nb * 512 : (nb + 1) * 512]
                        )
                        nc.sync.dma_start(
                            xi_d[m * 128 : (m + 1) * 128, nb * 512 : (nb + 1) * 512],
                            sb[:],
                        )

            # ---------- phase 2: recurrence over DAG levels ----------
            with (
                tc.tile_pool(name="rec", bufs=1) as rpool,
                tc.tile_pool(name="lvl", bufs=2) as lpool,
                tc.tile_pool(name="lvl3", bufs=3) as lpool3,
                tc.tile_pool(name="xiin", bufs=3) as xipool,
                tc.tile_pool(name="outp", bufs=1) as opool,
            ):
                whh = rpool.tile([128, KC * G3], f32r)  # 96 KB/partition
                nc.sync.dma_start(
                    whh[:].rearrange("p (k f) -> p k f", k=KC),
                    whh_d.rearrange("k p f -> p k f"),
                )
                bias2g = rpool.tile([128, H], f32)
                nc.sync.dma_start(bias2g[:], bias2_d[:])
                hist = rpool.tile([128, S_HIST * KC * BL], f32)

                for lv, nodes in enumerate(levels):
                    w = len(nodes)
                    M = BL * w
                    s0 = slot_of[nodes[0]]

                    # blended state, transposed layout, float32r
                    hbl = lpool3.tile([128, KC * WMAX * BL], f32r, tag="hbl")

                    # per-(chunk, node) blends: chunk-c strips complete as
                    # soon as chunk c's transpose lands, so MM K-chunk c of
                    # this level can start without waiting for chunks c+1..7
                    for c in range(KC):
                        for i, t in enumerate(nodes):
                            dst = hbl[:, c * WMAX * BL + i * BL : c * WMAX * BL + (i + 1) * BL]
                            a, b2 = int(d1[t]), int(d2[t])
                            if a < 0 and b2 < 0:
                                nc.gpsimd.tensor_copy(dst, zeros[:, c * 8 : c * 8 + 8])
                            elif a >= 0 and b2 >= 0:
                                ca = (slot_of[a] % S_HIST) * 64 + c * 8
                                cb = (slot_of[b2] % S_HIST) * 64 + c * 8
                                nc.gpsimd.tensor_add(
                                    dst, hist[:, ca : ca + 8], hist[:, cb : cb + 8]
                                )
                            else:
                                cs = (slot_of[a if a >= 0 else b2] % S_HIST) * 64 + c * 8
                                nc.gpsimd.tensor_copy(dst, hist[:, cs : cs + 8])

                    # xi rows for this level
                    xi = xipool.tile([WMAX * BL, G3], f32, tag="xi")
                    nc.sync.dma_start(xi[:M, :], xi_d[s0 * BL : s0 * BL + M, :])

                    # hh matmul: 6 banks x 8 K-chunks, N=512, fp32r
                    psb = []
                    for nb in range(NB):
                        ps = pspool.tile([128, 512], f32, tag="ps")
                        for k in range(KC):
                            nc.tensor.matmul(
                                ps[:M, :],
                                hbl_chunk(hbl[:], w, k),
                                whh[:, k * G3 + nb * 512 : k * G3 + (nb + 1) * 512],
                                start=(k == 0),
                                stop=(k == KC - 1),
                            )
                        psb.append(ps)

                    # hblend back to batch layout (needed for h_new formula)
                    hbb = lpool.tile([WMAX * BL, H], f32, tag="hbb")
                    for c in range(KC):
                        pt = ptpool.tile([128, 128], f32, tag="pt")
                        nc.tensor.transpose(
                            pt[:M, :128],
                            hbl_chunk(hbl[:], w, c).bitcast(f32),
                            ident[:128, :128],
                        )
                        nc.scalar.copy(hbb[:M, c * 128 : (c + 1) * 128], pt[:M, :128])

                    # gate banks are ordered [r0 r1 n0 n1 z0 z1] (xi/whh permuted)
                    rz = lpool.tile([WMAX * BL, 2 * H], f32, tag="rz")
                    for q in range(2):  # r
                        nc.vector.tensor_add(
                            rz[:M, q * 512 : (q + 1) * 512],
                            psb[q][:M, :],
                            xi[:M, q * 512 : (q + 1) * 512],
                        )
                        nc.scalar.activation(
                            rz[:M, q * 512 : (q + 1) * 512],
                            rz[:M, q * 512 : (q + 1) * 512],
                            AF.Sigmoid,
                        )
                    # n = tanh(xn + r * (hn + b_hh_n))   (banks 2..3)
                    # all tail ops split into 512-halves so transposes of the
                    # first half can start while the second half is in flight
                    npre = lpool.tile([WMAX * BL, H], f32, tag="npre")
                    for q in range(2):
                        hs = slice(q * 512, (q + 1) * 512)
                        nc.vector.tensor_add(
                            npre[:M, hs], psb[2 + q][:M, :], bias2g[:M, hs]
                        )
                        nc.vector.tensor_mul(npre[:M, hs], npre[:M, hs], rz[:M, hs])
                        nc.vector.tensor_add(
                            npre[:M, hs], npre[:M, hs], xi[:M, H + q * 512 : H + (q + 1) * 512]
                        )
                        nc.scalar.activation(npre[:M, hs], npre[:M, hs], AF.Tanh)
                    for q in range(2):  # z  (banks 4..5)
                        nc.vector.tensor_add(
                            rz[:M, H + q * 512 : H + (q + 1) * 512],
                            psb[4 + q][:M, :],
                            xi[:M, 2 * H + q * 512 : 2 * H + (q + 1) * 512],
                        )
                        nc.scalar.activation(
                            rz[:M, H + q * 512 : H + (q + 1) * 512],
                            rz[:M, H + q * 512 : H + (q + 1) * 512],
                            AF.Sigmoid,
                        )
                    # h_new = n + z*(hblend - n)   (in-place in hbb, per half)
                    for q in range(2):
                        hs = slice(q * 512, (q + 1) * 512)
                        zs = slice(H + q * 512, H + (q + 1) * 512)
                        nc.vector.tensor_sub(hbb[:M, hs], hbb[:M, hs], npre[:M, hs])
                        nc.vector.tensor_mul(hbb[:M, hs], hbb[:M, hs], rz[:M, zs])
                        nc.vector.tensor_add(hbb[:M, hs], hbb[:M, hs], npre[:M, hs])

                    # transpose h_new into the circular history store
                    for c in range(KC):
                        pt = ptpool.tile([128, 128], f32, tag="pt")
                        nc.tensor.transpose(
                            pt[:128, :M],
                            hbb[:M, c * 128 : (c + 1) * 128],
                            ident[:M, :M],
                        )
                        sm = s0 % S_HIST
                        n1 = min(w, S_HIST - sm)  # nodes before wrap
                        nc.scalar.copy(
                            hist_cols(hist[:], sm, n1, c),
                            pt[:, : n1 * BL].rearrange("p (n b) -> p n b", b=BL),
                        )
                        if n1 < w:
                            nc.scalar.copy(
                                hist_cols(hist[:], 0, w - n1, c),
                                pt[:, n1 * BL : M].rearrange("p (n b) -> p n b", b=BL),
                            )
                    if T - 1 in nodes:
                        # output: 2 * h_new(T-1), captured while the tile is live
                        i = nodes.index(T - 1)
                        outt = opool.tile([WMAX * BL, H], f32, tag="outt")
                        nc.vector.tensor_scalar_mul(outt[:M, :], hbb[:M, :], 2.0)
                        nc.sync.dma_start(out_d[:], outt[i * BL : (i + 1) * BL, :])

    nc.finalize()
    return nc


def kernel(**inputs):
    x = np.asarray(inputs["x"], np.float32)
    W_ih = np.asarray(inputs["W_ih"], np.float32)
    W_hh = np.asarray(inputs["W_hh"], np.float32)
    b_ih = np.asarray(inputs["b_ih"], np.float32)
    b_hh = np.asarray(inputs["b_hh"], np.float32)
    w1 = np.asarray(inputs["w1"], np.int32)
    w2 = np.asarray(inputs["w2"], np.int32)

    global WMAX
    slots_tmp, uz_tmp = _skip_plan(T, SKIP)
    wmax_needed = 0
    levels, order, slot_of, d1, d2 = _plan(w1, w2)
    wmax_needed = max(len(lv) for lv in levels)
    WMAX = max(9, wmax_needed)
    assert WMAX * BL <= 128, "level width exceeds one partition tile"
    nc = _build(levels, slot_of, d1, d2)

    perm = np.concatenate([np.arange(0, H), np.arange(2 * H, G3), np.arange(H, 2 * H)])
    W_hh = W_hh[perm]
    W_ih = W_ih[perm]
    bias = (b_ih + b_hh).copy()
    bias[2 * H :] = b_ih[2 * H :]  # n-part: only b_ih
    bias = bias[perm]
    whh_t = np.ascontiguousarray(W_hh.T.reshape(KC, 128, G3))
    wih_t = np.ascontiguousarray(W_ih.T.reshape(2, 128, G3))
    biasg = np.broadcast_to(bias, (128, G3)).copy()
    bias2g = np.broadcast_to(b_hh[2 * H :], (128, H)).copy()
    ident = np.eye(128, dtype=np.float32)
    in_maps = []
    for c in range(NCORES):
        xc = x[c * BL : (c + 1) * BL]  # [8, T, X]
        xsrt = xc[:, order, :]  # level-sorted
        xs = np.ascontiguousarray(xsrt.transpose(2, 1, 0).reshape(2, 128, T * BL))
        in_maps.append(
            {
                "xs": xs,
                "wih": wih_t,
                "whh": whh_t,
                "biasg": biasg,
                "bias2": bias2g,
                "ident": ident,
                "zeros": np.zeros((128, 64), np.float32),
            }
        )
    res = run_bass_kernel_spmd(nc, in_maps, core_ids=list(range(NCORES)))
    if getattr(res, "exec_time_ns", None):
        print("HW exec time:", res.exec_time_ns, "ns")
    global LAST_RESULT
    LAST_RESULT = res
    out = np.concatenate([res.results[c]["out"] for c in range(NCORES)], axis=0)
    return out.astype(np.float32)


LAST_RESULT = None


if __name__ == "__main__":
    rng = np.random.default_rng(0)
    ins = {
        "x": rng.standard_normal((B, T, X)).astype(np.float32),
        "W_ih": rng.standard_normal((G3, X)).astype(np.float32) / 32,
        "W_hh": rng.standard_normal((G3, H)).astype(np.float32) / 32,
        "b_ih": rng.standard_normal(G3).astype(np.float32) / 32,
        "b_hh": rng.standard_normal(G3).astype(np.float32) / 32,
        "w1": rng.integers(0, 2, T).astype(np.int32),
        "w2": rng.integers(0, 2, T).astype(np.int32),
        "skip_size": 5,
    }
    ins["w2"] = np.where(ins["w1"] == 0, 1, ins["w2"]).astype(np.int32)
    out = kernel(**ins)
    print("ran", out.shape, out.dtype, float(np.abs(out).mean()))


# revision 13
# speedup vs baseline: 1.0783x; 1.0783x over previous
"""Trainium2 Bass kernel for nn_Encoder_60112362275055 (GRU with skip connections).

B=64, T=512, X=256, H=1024, skip_size=5. Output = 2 * h_{T-1}  -> [64, 1024].

Strategy: data-parallel over batch (8 cores x B_local=8), ZERO cross-core
communication.  The skip structure (w1[t]==0 drops the h_{t-1} dependency)
turns the scan into a DAG whose critical path is ~88 topological levels
(avg width ~6).  All timesteps in one level are batched into a single
[8w, 1024] @ [1024, 3072] float32r matmul per core per level.
"""

import sys

import numpy as np

sys.path.insert(0, "/opt/trn_rl_repo")

import concourse.bacc as bacc
import concourse.bass as bass
import concourse.mybir as mybir
from concourse import tile
from concourse.bass_utils import run_bass_kernel_spmd

B, T, X, H = 64, 512, 256, 1024
SKIP = 5
NCORES = 8
BL = B // NCORES  # 8
G3 = 3 * H  # 3072
NB = G3 // 512  # 6 psum banks per level
KC = H // 128  # 8 K-chunks
WMAX = 9  # max level width (recomputed per input in kernel())
S_HIST = 96  # circular history slots

f32 = mybir.dt.float32
f32r = mybir.dt.float32r
AF = mybir.ActivationFunctionType


def _skip_plan(T, skip_size):
    slots = np.zeros(T, np.int32)
    use_zero = np.zeros(T, np.float32)
    for i in range(T):
        if i < skip_size:
            if 2 * i < skip_size:
                use_zero[i] = 1.0
            else:
                slots[i] = (skip_size - i) - 1
        else:
            if i - skip_size < skip_size:
                use_zero[i] = 1.0
            else:
                slots[i] = 2 * skip_size - 1
    return slots, use_zero


def _plan(w1, w2):
    """Topological levels of the recurrence DAG + per-node dependency spec."""
    slots, use_zero = _skip_plan(T, SKIP)
    d1 = np.full(T, -1, np.int64)  # h_{t-1} dep (coeff w1)
    d2 = np.full(T, -1, np.int64)  # skip dep (coeff w2*(1-uz))
    for t in range(T):
        if w1[t] == 1 and t - 1 >= 0:
            d1[t] = t - 1
        if w2[t] == 1 and use_zero[t] == 0.0 and t - 1 - slots[t] >= 0:
            d2[t] = t - 1 - slots[t]
    level = np.zeros(T, np.int64)
    for t in range(T):
        deps = [d for d in (d1[t], d2[t]) if d >= 0]
        level[t] = 1 + max((level[d] for d in deps), default=-1)
    nlev = int(level.max()) + 1
    levels = [[] for _ in range(nlev)]
    for t in range(T):
        levels[level[t]].append(t)
    order = [t for lv in levels for t in lv]
    slot_of = {t: s for s, t in enumerate(order)}

    # circular-history safety: the writer of slot s+S_HIST must come after the
    # last consumer of slot s.
    lev_of_slot = [level[order[s]] for s in range(T)]
    last_use = {}
    for t in range(T):
        for d in (d1[t], d2[t]):
            if d >= 0:
                s = slot_of[d]
                last_use[s] = max(last_use.get(s, -1), int(level[t]))
    for s in range(T):
        s2 = s + S_HIST
        if s2 < T and s in last_use:
            assert lev_of_slot[s2] > last_use[s], (s, s2, "S_HIST too small")
    return levels, order, slot_of, d1, d2


def _build(levels, slot_of, d1, d2):
    """Emit the bass program (identical on all 8 cores)."""
    nc = bacc.Bacc(None)

    xs_d = nc.dram_tensor("xs", [2, 128, T * BL], f32r, kind="ExternalInput")
    wih_d = nc.dram_tensor("wih", [2, 128, G3], f32r, kind="ExternalInput")
    whh_d = nc.dram_tensor("whh", [KC, 128, G3], f32r, kind="ExternalInput")
    biasg_d = nc.dram_tensor("biasg", [128, G3], f32, kind="ExternalInput")
    bias2_d = nc.dram_tensor("bias2", [128, H], f32, kind="ExternalInput")
    ident_d = nc.dram_tensor("ident", [128, 128], f32, kind="ExternalInput")
    zeros_d = nc.dram_tensor("zeros", [128, 64], f32r, kind="ExternalInput")
    out_d = nc.dram_tensor("out", [BL, H], f32, kind="ExternalOutput")
    xi_d = nc.dram_tensor("xi_scratch", [T * BL, G3], f32)

    NLEV = len(levels)

    def hbl_chunk(hbl_ap, w, c):
        """lhsT for K-chunk c (chunk-major layout): contiguous [128, w*8]."""
        return hbl_ap[:, c * WMAX * BL : c * WMAX * BL + w * BL]

    def hbl_node(hbl_ap, i):
        """blend dst for node i: strided [128, KC, BL]."""
        return hbl_ap.rearrange("p (c n b) -> p c n b", n=WMAX, b=BL)[:, :, i, :]

    def hist_cols(hist_ap, sm, w, c):
        """history store cols (sm+i)*64 + c*8 + b -> [128, w, 8] (no wrap)."""
        return hist_ap.rearrange("p (s c b) -> p s c b", c=KC, b=BL)[:, sm : sm + w, c, :]

    with tile.TileContext(nc) as tc:
        with (
            tc.tile_pool(name="consts", bufs=1) as cpool,
            tc.tile_pool(name="ps", bufs=6, space="PSUM") as pspool,
            tc.tile_pool(name="pst", bufs=2, space="PSUM") as ptpool,
        ):
            ident = cpool.tile([128, 128], f32)
            nc.sync.dma_start(ident[:], ident_d[:])
            zeros = cpool.tile([128, 64], f32r)
            nc.sync.dma_start(zeros[:], zeros_d[:])

            # ---------- phase 1: xi = x @ W_ih.T + bias (level-sorted rows) ----------
            with tc.tile_pool(name="ph1", bufs=1) as p1, tc.tile_pool(name="ph1b", bufs=3) as p1b:
                wih = p1.tile([128, 2 * G3], f32r)
                nc.sync.dma_start(
                    wih[:].rearrange("p (k f) -> p k f", k=2),
                    wih_d.rearrange("k p f -> p k f"),
                )
                xs = p1.tile([128, 2 * T * BL], f32r)
                nc.sync.dma_start(
                    xs[:].rearrange("p (k f) -> p k f", k=2),
                    xs_d.rearrange("k p f -> p k f"),
                )
                biasg = p1.tile([128, G3], f32)
                nc.sync.dma_start(biasg[:], biasg_d[:])
                MT = T * BL // 128  # 32 M-tiles
                for m in range(MT):
                    for nb in range(NB):
                        ps = pspool.tile([128, 512], f32, tag="ps")
                        for k in range(2):
                            nc.tensor.matmul(
                                ps[:],
                                xs[:, k * T * BL + m * 128 : k * T * BL + (m + 1) * 128],
                                wih[:, k * G3 + nb * 512 : k * G3 + (nb + 1) * 512],
                                start=(k == 0),
                                stop=(k == 1),
                            )
                        sb = p1b.tile([128, 512], f32, tag="xsb")
                        nc.vector.tensor_add(
                            sb[:], ps[:], biasg[:, nb * 512 : (nb + 1) * 512]
                        )
                        nc.sync.dma_start(
                            xi_d[m * 128 : (m + 1) * 128, nb * 512 : (nb + 1) * 512],
                            sb[:],
                        )

            # ---------- phase 2: recurrence over DAG levels ----------
            with (
                tc.tile_pool(name="rec", bufs=1) as rpool,
                tc.tile_pool(name="lvl", bufs=2) as lpool,
                tc.tile_pool(name="lvl3", bufs=3) as lpool3,
                tc.tile_pool(name="xiin", bufs=3) as xipool,
                tc.tile_pool(name="outp", bufs=1) as opool,
            ):
                whh = rpool.tile([128, KC * G3], f32r)  # 96 KB/partition
                nc.sync.dma_start(
                    whh[:].rearrange("p (k f) -> p k f", k=KC),
                    whh_d.rearrange("k p f -> p k f"),
                )
                bias2g = rpool.tile([128, H], f32)
                nc.sync.dma_start(bias2g[:], bias2_d[:])
                hist = rpool.tile([128, S_HIST * KC * BL], f32)

                for lv, nodes in enumerate(levels):
                    w = len(nodes)
                    M = BL * w
                    s0 = slot_of[nodes[0]]

                    # blended state, transposed layout, float32r
                    hbl = lpool3.tile([128, KC * WMAX * BL], f32r, tag="hbl")

                    def hist_node(s):
                        col = (s % S_HIST) * 64
                        return hist[:, col : col + 64].rearrange(
                            "p (c b) -> p c b", b=BL
                        )

                    for i, t in enumerate(nodes):
                        dst = hbl_node(hbl[:], i)
                        a, b2 = int(d1[t]), int(d2[t])
                        if a < 0 and b2 < 0:
                            nc.scalar.copy(
                                dst, zeros[:].rearrange("p (c b) -> p c b", b=BL)
                            )
                        elif a >= 0 and b2 >= 0:
                            nc.gpsimd.tensor_add(
                                dst, hist_node(slot_of[a]), hist_node(slot_of[b2])
                            )
                        else:
                            nc.scalar.copy(
                                dst, hist_node(slot_of[a if a >= 0 else b2])
                            )

                    # xi rows for this level
                    xi = xipool.tile([WMAX * BL, G3], f32, tag="xi")
                    nc.sync.dma_start(xi[:M, :], xi_d[s0 * BL : s0 * BL + M, :])

                    # hh matmul: 6 banks x 8 K-chunks, N=512, fp32r
                    psb = []
                    for nb in range(NB):
                        ps = pspool.tile([128, 512], f32, tag="ps")
                        for k in range(KC):
                            nc.tensor.matmul(
                                ps[:M, :],
                                hbl_chunk(hbl[:], w, k),
                                whh[:, k * G3 + nb * 512 : k * G3 + (nb + 1) * 512],
                                start=(k == 0),
                                stop=(k == KC - 1),
                            )
                        psb.append(ps)

                    # hblend back to batch layout (needed for h_new formula)
                    hbb = lpool.tile([WMAX * BL, H], f32, tag="hbb")
                    for c in range(KC):
                        pt = ptpool.tile([128, 128], f32, tag="pt")
                        nc.tensor.transpose(
                            pt[:M, :128],
                            hbl_chunk(hbl[:], w, c).bitcast(f32),
                            ident[:128, :128],
                        )
                        nc.scalar.copy(hbb[:M, c * 128 : (c + 1) * 128], pt[:M, :128])

                    # gate banks are ordered [r0 r1 n0 n1 z0 z1] (xi/whh permuted)
                    rz = lpool.tile([WMAX * BL, 2 * H], f32, tag="rz")
                    for q in range(2):  # r
                        nc.vector.tensor_add(
                            rz[:M, q * 512 : (q + 1) * 512],
                            psb[q][:M, :],
                            xi[:M, q * 512 : (q + 1) * 512],
                        )
                        nc.scalar.activation(
                            rz[:M, q * 512 : (q + 1) * 512],
                            rz[:M, q * 512 : (q + 1) * 512],
                            AF.Sigmoid,
                        )
                    # n = tanh(xn + r * (hn + b_hh_n))   (banks 2..3)
                    # all tail ops split into 512-halves so transposes of the
                    # first half can start while the second half is in flight
                    npre = lpool.tile([WMAX * BL, H], f32, tag="npre")
                    for q in range(2):
                        hs = slice(q * 512, (q + 1) * 512)
                        nc.vector.tensor_add(
                            npre[:M, hs], psb[2 + q][:M, :], bias2g[:M, hs]
                        )
                        nc.vector.tensor_mul(npre[:M, hs], npre[:M, hs], rz[:M, hs])
                        nc.vector.tensor_add(
                            npre[:M, hs], npre[:M, hs], xi[:M, H + q * 512 : H + (q + 1) * 512]
                        )
                        nc.scalar.activation(npre[:M, hs], npre[:M, hs], AF.Tanh)
                    for q in range(2):  # z  (banks 4..5)
                        nc.vector.tensor_add(
                            rz[:M, H + q * 512 : H + (q + 1) * 512],
                            psb[4 + q][:M, :],
                            xi[:M, 2 * H + q * 512 : 2 * H + (q + 1) * 512],
                        )
                        nc.scalar.activation(
                            rz[:M, H + q * 512 : H + (q + 1) * 512],
                            rz[:M, H + q * 512 : H + (q + 1) * 512],
                            AF.Sigmoid,
                        )
                    # h_new = n + z*(hblend - n)   (in-place in hbb, per half)
                    for q in range(2):
                        hs = slice(q * 512, (q + 1) * 512)
                        zs = slice(H + q * 512, H + (q + 1) * 512)
                        nc.vector.tensor_sub(hbb[:M, hs], hbb[:M, hs], npre[:M, hs])
                        nc.vector.tensor_mul(hbb[:M, hs], hbb[:M, hs], rz[:M, zs])
                        nc.vector.tensor_add(hbb[:M, hs], hbb[:M, hs], npre[:M, hs])

                    # transpose h_new into the circular history store
                    for c in range(KC):
                        pt = ptpool.tile([128, 128], f32, tag="pt")
                        nc.tensor.transpose(
                            pt[:128, :M],
                            hbb[:M, c * 128 : (c + 1) * 128],
                            ident[:M, :M],
                        )
                        sm = s0 % S_HIST
                        n1 = min(w, S_HIST - sm)  # nodes before wrap
                        nc.scalar.copy(
                            hist_cols(hist[:], sm, n1, c),
                            pt[:, : n1 * BL].rearrange("p (n b) -> p n b", b=BL),
                        )
                        if n1 < w:
                            nc.scalar.copy(
                                hist_cols(hist[:], 0, w - n1, c),
                                pt[:, n1 * BL : M].rearrange("p (n b) -> p n b", b=BL),
                            )
                    if T - 1 in nodes:
                        # output: 2 * h_new(T-1), captured while the tile is live
                        i = nodes.index(T - 1)
                        outt = opool.tile([WMAX * BL, H], f32, tag="outt")
                        nc.vector.tensor_scalar_mul(outt[:M, :], hbb[:M, :], 2.0)
                        nc.sync.dma_start(out_d[:], outt[i * BL : (i + 1) * BL, :])

    nc.finalize()
    return nc


def kernel(**inputs):
    x = np.asarray(inputs["x"], np.float32)
    W_ih = np.asarray(inputs["W_ih"], np.float32)
    W_hh = np.asarray(inputs["W_hh"], np.float32)
    b_ih = np.asarray(inputs["b_ih"], np.float32)
    b_hh = np.asarray(inputs["b_hh"], np.float32)
    w1 = np.asarray(inputs["w1"], np.int32)
    w2 = np.asarray(inputs["w2"], np.int32)

    global WMAX
    slots_tmp, uz_tmp = _skip_plan(T, SKIP)
    wmax_needed = 0
    levels, order, slot_of, d1, d2 = _plan(w1, w2)
    wmax_needed = max(len(lv) for lv in levels)
    WMAX = max(9, wmax_needed)
    assert WMAX * BL <= 128, "level width exceeds one partition tile"
    nc = _build(levels, slot_of, d1, d2)

    perm = np.concatenate([np.arange(0, H), np.arange(2 * H, G3), np.arange(H, 2 * H)])
    W_hh = W_hh[perm]
    W_ih = W_ih[perm]
    bias = (b_ih + b_hh).copy()
    bias[2 * H :] = b_ih[2 * H :]  # n-part: only b_ih
    bias = bias[perm]
    whh_t = np.ascontiguousarray(W_hh.T.reshape(KC, 128, G3))
    wih_t = np.ascontiguousarray(W_ih.T.reshape(2, 128, G3))
    biasg = np.broadcast_to(bias, (128, G3)).copy()
    bias2g = np.broadcast_to(b_hh[2 * H :], (128, H)).copy()
    ident = np.eye(128, dtype=np.float32)
    in_maps = []
    for c in range(NCORES):
        xc = x[c * BL : (c + 1) * BL]  # [8, T, X]
        xsrt = xc[:, order, :]  # level-sorted
        xs = np.ascontiguousarray(xsrt.transpose(2, 1, 0).reshape(2, 128, T * BL))
        in_maps.append(
            {
                "xs": xs,
                "wih": wih_t,
                "whh": whh_t,
                "biasg": biasg,
                "bias2": bias2g,
                "ident": ident,
                "zeros": np.zeros((128, 64), np.float32),
            }
        )
    res = run_bass_kernel_spmd(nc, in_maps, core_ids=list(range(NCORES)))
    if getattr(res, "exec_time_ns", None):
        print("HW exec time:", res.exec_time_ns, "ns")
    global LAST_RESULT
    LAST_RESULT = res
    out = np.concatenate([res.results[c]["out"] for c in range(NCORES)], axis=0)
    return out.astype(np.float32)


LAST_RESULT = None


if __name__ == "__main__":
    rng = np.random.default_rng(0)
    ins = {
        "x": rng.standard_normal((B, T, X)).astype(np.float32),
        "W_ih": rng.standard_normal((G3, X)).astype(np.float32) / 32,
        "W_hh": rng.standard_normal((G3, H)).astype(np.float32) / 32,
        "b_ih": rng.standard_normal(G3).astype(np.float32) / 32,
        "b_hh": rng.standard_normal(G3).astype(np.float32) / 32,
        "w1": rng.integers(0, 2, T).astype(np.int32),
        "w2": rng.integers(0, 2, T).astype(np.int32),
        "skip_size": 5,
    }
    ins["w2"] = np.where(ins["w1"] == 0, 1, ins["w2"]).astype(np.int32)
    out = kernel(**ins)
    print("ran", out.shape, out.dtype, float(np.abs(out).mean()))


# revision 14
# speedup vs baseline: 1.1529x; 1.0692x over previous
"""Trainium2 Bass kernel for nn_Encoder_60112362275055 (GRU with skip connections).

B=64, T=512, X=256, H=1024, skip_size=5. Output = 2 * h_{T-1}  -> [64, 1024].

Strategy: data-parallel over batch (8 cores x B_local=8), ZERO cross-core
communication.  The skip structure (w1[t]==0 drops the h_{t-1} dependency)
turns the scan into a DAG whose critical path is ~88 topological levels
(avg width ~6).  All timesteps in one level are batched into a single
[8w, 1024] @ [1024, 3072] float32r matmul per core per level.
"""

import sys

import numpy as np

sys.path.insert(0, "/opt/trn_rl_repo")

import concourse.bacc as bacc
import concourse.bass as bass
import concourse.mybir as mybir
from concourse import tile
from concourse.bass_utils import run_bass_kernel_spmd

B, T, X, H = 64, 512, 256, 1024
SKIP = 5
NCORES = 8
BL = B // NCORES  # 8
G3 = 3 * H  # 3072
NB = G3 // 512  # 6 psum banks per level
KC = H // 128  # 8 K-chunks
WMAX = 9  # max level width (recomputed per input in kernel())
S_HIST = 96  # circular history slots

f32 = mybir.dt.float32
f32r = mybir.dt.float32r
AF = mybir.ActivationFunctionType


def _skip_plan(T, skip_size):
    slots = np.zeros(T, np.int32)
    use_zero = np.zeros(T, np.float32)
    for i in range(T):
        if i < skip_size:
            if 2 * i < skip_size:
                use_zero[i] = 1.0
            else:
                slots[i] = (skip_size - i) - 1
        else:
            if i - skip_size < skip_size:
                use_zero[i] = 1.0
            else:
                slots[i] = 2 * skip_size - 1
    return slots, use_zero


def _plan(w1, w2):
    """Topological levels of the recurrence DAG + per-node dependency spec."""
    slots, use_zero = _skip_plan(T, SKIP)
    d1 = np.full(T, -1, np.int64)  # h_{t-1} dep (coeff w1)
    d2 = np.full(T, -1, np.int64)  # skip dep (coeff w2*(1-uz))
    for t in range(T):
        if w1[t] == 1 and t - 1 >= 0:
            d1[t] = t - 1
        if w2[t] == 1 and use_zero[t] == 0.0 and t - 1 - slots[t] >= 0:
            d2[t] = t - 1 - slots[t]
    level = np.zeros(T, np.int64)
    for t in range(T):
        deps = [d for d in (d1[t], d2[t]) if d >= 0]
        level[t] = 1 + max((level[d] for d in deps), default=-1)
    nlev = int(level.max()) + 1
    levels = [[] for _ in range(nlev)]
    for t in range(T):
        levels[level[t]].append(t)
    order = [t for lv in levels for t in lv]
    slot_of = {t: s for s, t in enumerate(order)}

    # circular-history safety: the writer of slot s+S_HIST must come after the
    # last consumer of slot s.
    lev_of_slot = [level[order[s]] for s in range(T)]
    last_use = {}
    for t in range(T):
        for d in (d1[t], d2[t]):
            if d >= 0:
                s = slot_of[d]
                last_use[s] = max(last_use.get(s, -1), int(level[t]))
    for s in range(T):
        s2 = s + S_HIST
        if s2 < T and s in last_use:
            assert lev_of_slot[s2] > last_use[s], (s, s2, "S_HIST too small")
    return levels, order, slot_of, d1, d2


def _build(levels, slot_of, d1, d2):
    """Emit the bass program (identical on all 8 cores)."""
    nc = bacc.Bacc(None)

    xs_d = nc.dram_tensor("xs", [2, 128, T * BL], f32r, kind="ExternalInput")
    wih_d = nc.dram_tensor("wih", [2, 128, G3], f32r, kind="ExternalInput")
    whh_d = nc.dram_tensor("whh", [KC, 128, G3], f32r, kind="ExternalInput")
    biasg_d = nc.dram_tensor("biasg", [128, G3], f32, kind="ExternalInput")
    bias2_d = nc.dram_tensor("bias2", [128, H], f32, kind="ExternalInput")
    ident_d = nc.dram_tensor("ident", [128, 128], f32, kind="ExternalInput")
    zeros_d = nc.dram_tensor("zeros", [128, 64], f32r, kind="ExternalInput")
    out_d = nc.dram_tensor("out", [BL, H], f32, kind="ExternalOutput")
    xi_d = nc.dram_tensor("xi_scratch", [T * BL, G3], f32)

    NLEV = len(levels)

    def hbl_chunk(hbl_ap, w, c):
        """lhsT for K-chunk c (chunk-major layout): contiguous [128, w*8]."""
        return hbl_ap[:, c * WMAX * BL : c * WMAX * BL + w * BL]

    def hbl_node(hbl_ap, i):
        """blend dst for node i: strided [128, KC, BL]."""
        return hbl_ap.rearrange("p (c n b) -> p c n b", n=WMAX, b=BL)[:, :, i, :]

    def hist_cols(hist_ap, sm, w, c):
        """history store cols (sm+i)*64 + c*8 + b -> [128, w, 8] (no wrap)."""
        return hist_ap.rearrange("p (s c b) -> p s c b", c=KC, b=BL)[:, sm : sm + w, c, :]

    with tile.TileContext(nc) as tc:
        with (
            tc.tile_pool(name="consts", bufs=1) as cpool,
            tc.tile_pool(name="ps", bufs=6, space="PSUM") as pspool,
            tc.tile_pool(name="pst", bufs=2, space="PSUM") as ptpool,
        ):
            ident = cpool.tile([128, 128], f32)
            nc.sync.dma_start(ident[:], ident_d[:])
            zeros = cpool.tile([128, 64], f32r)
            nc.sync.dma_start(zeros[:], zeros_d[:])

            # ---------- phase 1: xi = x @ W_ih.T + bias (level-sorted rows) ----------
            with tc.tile_pool(name="ph1", bufs=1) as p1, tc.tile_pool(name="ph1b", bufs=3) as p1b:
                wih = p1.tile([128, 2 * G3], f32r)
                nc.sync.dma_start(
                    wih[:].rearrange("p (k f) -> p k f", k=2),
                    wih_d.rearrange("k p f -> p k f"),
                )
                xs = p1.tile([128, 2 * T * BL], f32r)
                nc.sync.dma_start(
                    xs[:].rearrange("p (k f) -> p k f", k=2),
                    xs_d.rearrange("k p f -> p k f"),
                )
                biasg = p1.tile([128, G3], f32)
                nc.sync.dma_start(biasg[:], biasg_d[:])
                MT = T * BL // 128  # 32 M-tiles
                for m in range(MT):
                    for nb in range(NB):
                        ps = pspool.tile([128, 512], f32, tag="ps")
                        for k in range(2):
                            nc.tensor.matmul(
                                ps[:],
                                xs[:, k * T * BL + m * 128 : k * T * BL + (m + 1) * 128],
                                wih[:, k * G3 + nb * 512 : k * G3 + (nb + 1) * 512],
                                start=(k == 0),
                                stop=(k == 1),
                            )
                        sb = p1b.tile([128, 512], f32, tag="xsb")
                        nc.vector.tensor_add(
                            sb[:], ps[:], biasg[:, nb * 512 : (nb + 1) * 512]
                        )
                        nc.sync.dma_start(
                            xi_d[m * 128 : (m + 1) * 128, nb * 512 : (nb + 1) * 512],
                            sb[:],
                        )

            # ---------- phase 2: recurrence over DAG levels ----------
            with (
                tc.tile_pool(name="rec", bufs=1) as rpool,
                tc.tile_pool(name="lvl", bufs=2) as lpool,
                tc.tile_pool(name="lvl3", bufs=3) as lpool3,
                tc.tile_pool(name="xiin", bufs=3) as xipool,
                tc.tile_pool(name="outp", bufs=1) as opool,
            ):
                whh = rpool.tile([128, KC * G3], f32r)  # 96 KB/partition
                nc.sync.dma_start(
                    whh[:].rearrange("p (k f) -> p k f", k=KC),
                    whh_d.rearrange("k p f -> p k f"),
                )
                bias2g = rpool.tile([128, H], f32)
                nc.sync.dma_start(bias2g[:], bias2_d[:])
                hist = rpool.tile([128, S_HIST * KC * BL], f32)

                for lv, nodes in enumerate(levels):
                    w = len(nodes)
                    M = BL * w
                    s0 = slot_of[nodes[0]]

                    # blended state, transposed layout, float32r
                    hbl = lpool3.tile([128, KC * WMAX * BL], f32r, tag="hbl")

                    def hist_node(s):
                        col = (s % S_HIST) * 64
                        return hist[:, col : col + 64].rearrange(
                            "p (c b) -> p c b", b=BL
                        )

                    for i, t in enumerate(nodes):
                        dst = hbl_node(hbl[:], i)
                        a, b2 = int(d1[t]), int(d2[t])
                        if a < 0 and b2 < 0:
                            nc.scalar.copy(
                                dst, zeros[:].rearrange("p (c b) -> p c b", b=BL)
                            )
                        elif a >= 0 and b2 >= 0:
                            nc.gpsimd.tensor_add(
                                dst, hist_node(slot_of[a]), hist_node(slot_of[b2])
                            )
                        else:
                            nc.scalar.copy(
                                dst, hist_node(slot_of[a if a >= 0 else b2])
                            )

                    # xi rows for this level
                    xi = xipool.tile([WMAX * BL, G3], f32, tag="xi")
                    nc.sync.dma_start(xi[:M, :], xi_d[s0 * BL : s0 * BL + M, :])

                    # hh matmul: 6 banks x 8 K-chunks, N=512, fp32r
                    psb = []
                    for nb in range(NB):
                        ps = pspool.tile([128, 512], f32, tag="ps")
                        for k in range(KC):
                            nc.tensor.matmul(
                                ps[:M, :],
                                hbl_chunk(hbl[:], w, k),
                                whh[:, k * G3 + nb * 512 : k * G3 + (nb + 1) * 512],
                                start=(k == 0),
                                stop=(k == KC - 1),
                            )
                        psb.append(ps)

                    # hblend back to batch layout (needed for h_new formula)
                    hbb = lpool.tile([WMAX * BL, H], f32, tag="hbb")
                    for c in range(KC):
                        pt = ptpool.tile([128, 128], f32, tag="pt")
                        nc.tensor.transpose(
                            pt[:M, :128],
                            hbl_chunk(hbl[:], w, c).bitcast(f32),
                            ident[:128, :128],
                        )
                        nc.scalar.copy(hbb[:M, c * 128 : (c + 1) * 128], pt[:M, :128])

                    # gate banks are ordered [r0 r1 n0 n1 z0 z1] (xi/whh permuted)
                    rz = lpool.tile([WMAX * BL, 2 * H], f32, tag="rz")
                    for q in range(2):  # r
                        nc.vector.tensor_add(
                            rz[:M, q * 512 : (q + 1) * 512],
                            psb[q][:M, :],
                            xi[:M, q * 512 : (q + 1) * 512],
                        )
                        nc.scalar.activation(
                            rz[:M, q * 512 : (q + 1) * 512],
                            rz[:M, q * 512 : (q + 1) * 512],
                            AF.Sigmoid,
                        )
                    # n = tanh(xn + r * (hn + b_hh_n))   (banks 2..3)
                    # all tail ops split into 512-halves so transposes of the
                    # first half can start while the second half is in flight
                    npre = lpool.tile([WMAX * BL, H], f32, tag="npre")
                    for q in range(4):
                        hs = slice(q * 256, (q + 1) * 256)
                        nc.vector.tensor_add(
                            npre[:M, hs],
                            psb[2 + q // 2][:M, (q % 2) * 256 : (q % 2) * 256 + 256],
                            bias2g[:M, hs],
                        )
                        nc.vector.tensor_mul(npre[:M, hs], npre[:M, hs], rz[:M, hs])
                        nc.vector.tensor_add(
                            npre[:M, hs], npre[:M, hs], xi[:M, H + q * 256 : H + (q + 1) * 256]
                        )
                        nc.scalar.activation(npre[:M, hs], npre[:M, hs], AF.Tanh)
                    for q in range(2):  # z  (banks 4..5)
                        nc.vector.tensor_add(
                            rz[:M, H + q * 512 : H + (q + 1) * 512],
                            psb[4 + q][:M, :],
                            xi[:M, 2 * H + q * 512 : 2 * H + (q + 1) * 512],
                        )
                        nc.scalar.activation(
                            rz[:M, H + q * 512 : H + (q + 1) * 512],
                            rz[:M, H + q * 512 : H + (q + 1) * 512],
                            AF.Sigmoid,
                        )
                    # h_new = n + z*(hblend - n)   (in-place in hbb, per half)
                    for q in range(4):
                        hs = slice(q * 256, (q + 1) * 256)
                        zs = slice(H + q * 256, H + (q + 1) * 256)
                        nc.vector.tensor_sub(hbb[:M, hs], hbb[:M, hs], npre[:M, hs])
                        nc.vector.tensor_mul(hbb[:M, hs], hbb[:M, hs], rz[:M, zs])
                        nc.vector.tensor_add(hbb[:M, hs], hbb[:M, hs], npre[:M, hs])

                    # transpose h_new into the circular history store
                    for c in range(KC):
                        pt = ptpool.tile([128, 128], f32, tag="pt")
                        nc.tensor.transpose(
                            pt[:128, :M],
                            hbb[:M, c * 128 : (c + 1) * 128],
                            ident[:M, :M],
                        )
                        sm = s0 % S_HIST
                        n1 = min(w, S_HIST - sm)  # nodes before wrap
                        nc.scalar.copy(
                            hist_cols(hist[:], sm, n1, c),
                            pt[:, : n1 * BL].rearrange("p (n b) -> p n b", b=BL),
                        )
                        if n1 < w:
                            nc.scalar.copy(
                                hist_cols(hist[:], 0, w - n1, c),
                                pt[:, n1 * BL : M].rearrange("p (n b) -> p n b", b=BL),
                            )
                    if T - 1 in nodes:
                        # output: 2 * h_new(T-1), captured while the tile is live
                        i = nodes.index(T - 1)
                        outt = opool.tile([WMAX * BL, H], f32, tag="outt")
                        nc.vector.tensor_scalar_mul(outt[:M, :], hbb[:M, :], 2.0)
                        nc.sync.dma_start(out_d[:], outt[i * BL : (i + 1) * BL, :])

    nc.finalize()
    return nc


def kernel(**inputs):
    x = np.asarray(inputs["x"], np.float32)
    W_ih = np.asarray(inputs["W_ih"], np.float32)
    W_hh = np.asarray(inputs["W_hh"], np.float32)
    b_ih = np.asarray(inputs["b_ih"], np.float32)
    b_hh = np.asarray(inputs["b_hh"], np.float32)
    w1 = np.asarray(inputs["w1"], np.int32)
    w2 = np.asarray(inputs["w2"], np.int32)

    global WMAX
    slots_tmp, uz_tmp = _skip_plan(T, SKIP)
    wmax_needed = 0
    levels, order, slot_of, d1, d2 = _plan(w1, w2)
    wmax_needed = max(len(lv) for lv in levels)
    WMAX = max(9, wmax_needed)
    assert WMAX * BL <= 128, "level width exceeds one partition tile"
    nc = _build(levels, slot_of, d1, d2)

    perm = np.concatenate([np.arange(0, H), np.arange(2 * H, G3), np.arange(H, 2 * H)])
    W_hh = W_hh[perm]
    W_ih = W_ih[perm]
    bias = (b_ih + b_hh).copy()
    bias[2 * H :] = b_ih[2 * H :]  # n-part: only b_ih
    bias = bias[perm]
    whh_t = np.ascontiguousarray(W_hh.T.reshape(KC, 128, G3))
    wih_t = np.ascontiguousarray(W_ih.T.reshape(2, 128, G3))
    biasg = np.broadcast_to(bias, (128, G3)).copy()
    bias2g = np.broadcast_to(b_hh[2 * H :], (128, H)).copy()
    ident = np.eye(128, dtype=np.float32)
    in_maps = []
    for c in range(NCORES):
        xc = x[c * BL : (c + 1) * BL]  # [8, T, X]
        xsrt = xc[:, order, :]  # level-sorted
        xs = np.ascontiguousarray(xsrt.transpose(2, 1, 0).reshape(2, 128, T * BL))
        in_maps.append(
            {
                "xs": xs,
                "wih": wih_t,
                "whh": whh_t,
                "biasg": biasg,
                "bias2": bias2g,
                "ident": ident,
                "zeros": np.zeros((128, 64), np.float32),
            }
        )
    res = run_bass_kernel_spmd(nc, in_maps, core_ids=list(range(NCORES)))
    if getattr(res, "exec_time_ns", None):
        print("HW exec time:", res.exec_time_ns, "ns")
    global LAST_RESULT
    LAST_RESULT = res
    out = np.concatenate([res.results[c]["out"] for c in range(NCORES)], axis=0)
    return out.astype(np.float32)


LAST_RESULT = None


if __name__ == "__main__":
    rng = np.random.default_rng(0)
    ins = {
        "x": rng.standard_normal((B, T, X)).astype(np.float32),
        "W_ih": rng.standard_normal((G3, X)).astype(np.float32) / 32,
        "W_hh": rng.standard_normal((G3, H)).astype(np.float32) / 32,
        "b_ih": rng.standard_normal(G3).astype(np.float32) / 32,
        "b_hh": rng.standard_normal(G3).astype(np.float32) / 32,
        "w1": rng.integers(0, 2, T).astype(np.int32),
        "w2": rng.integers(0, 2, T).astype(np.int32),
        "skip_size": 5,
    }
    ins["w2"] = np.where(ins["w1"] == 0, 1, ins["w2"]).astype(np.int32)
    out = kernel(**ins)
    print("ran", out.shape, out.dtype, float(np.abs(out).mean()))
